# revision 14
# baseline (speedup 1.0000x reference)
"""Causal self-attention (GQA + RMS-norm + RoPE) Trainium2 Bass kernel.

Sharding over 8 NeuronCores: 2-way data parallel (batch) x 4-way head
parallel (one GQA group of 4 q-heads + 1 kv-head per core).  Each core
computes q/k/v projections for its group, flash-style causal attention
(scores kept transposed [k, q] so softmax sums ride the tensor engine),
and a partial output projection.  Host sums the 4 per-group partials per
batch.

All matmul operands are bf16 (fp32 accumulation in PSUM).  RMS-norm
bounds |scores| <= gain*sqrt(hd), so softmax needs no max-subtraction.

Perf structure (v4):
- All DRAM tensors are [128, *] partition-major; loads are full-width
  single DMAs issued k/v-weights first, then the 16 x dt-tiles; the k/v
  projection wave is dt-interleaved across 5 PSUM banks so the PE rides
  the x DMA wavefront.
- ScalarE uses only {Copy, Square, Ln, Exp} from one table set
  (natural_log_exp_and_others): rsqrt = exp(-0.5*ln(.)).  No table
  reloads -> no PE cold windows.
- Attention scores for two adjacent k-tiles share one [128,1024] PSUM
  tile and one ACTIVATE (exp); causal masking uses precomputed
  staircase masks; the softmax denominator matmul runs once per FOUR
  k-tiles on the DVE-summed P (quarters its PE cost).
- Main loop is q-block-outer: per block j, the 4 q-head norm chains,
  then the 4 attention blocks with output-projection tiles of column
  j-1 woven in as PE filler (PE stays busy while ACT chews exp), then
  8 more filler closures are queued.  Norm tails drain one-per-block.
- Norm/RoPE pipeline runs bf16 end-to-end (2x DVE rate).
- PSUM stale-data invariant: the "big" tag only ever holds bounded
  pre-activation values (|.| < 12), so exp of a stale region (later
  masked to zero) cannot overflow.  acc_y / acc_d (unbounded sums)
  live on their own tags and are never exp'd.
"""

import numpy as np
import ml_dtypes

import concourse.bacc as bacc
import concourse.mybir as mybir
from concourse.tile import TileContext
from concourse.bass_utils import run_bass_kernel_spmd

BF16 = mybir.dt.bfloat16
F32 = mybir.dt.float32
F32R = mybir.dt.float32r
AF = mybir.ActivationFunctionType
bf = ml_dtypes.bfloat16

B, S, D = 2, 2048, 2048
H, HKV, HD = 16, 4, 128
RQ = H // HKV            # q heads per kv group (4)
NCORES = 8
NDT = D // 128           # 16 contraction tiles
NST = S // 512           # 4 query/sequence 512-tiles
NKT = S // 128           # 16 key 128-tiles
EPS = float(np.finfo(np.float32).eps)

_PROG_CACHE = {}


def _build_program(n_timing_iters=1):
    nc = bacc.Bacc("TRN2", debug=False, enable_asserts=False, num_devices=NCORES)

    xT_d = nc.dram_tensor("xT", [128, NDT * S], BF16, kind="ExternalInput")
    # head-major: head h occupies cols [h*NDT*HD, (h+1)*NDT*HD), dt-strided
    wqT_d = nc.dram_tensor("wqT", [128, RQ * NDT * HD], BF16, kind="ExternalInput")
    wkT_d = nc.dram_tensor("wkT", [128, NDT * HD], BF16, kind="ExternalInput")
    wvT_d = nc.dram_tensor("wvT", [128, NDT * HD], BF16, kind="ExternalInput")
    # dt-major: dt block cols [dt*RQ*128, (dt+1)*RQ*128), h within
    wpT_d = nc.dram_tensor("wpT", [128, NDT * RQ * 128], BF16, kind="ExternalInput")
    cosF_d = nc.dram_tensor("cosF", [128, S], BF16, kind="ExternalInput")
    sinF_d = nc.dram_tensor("sinF", [128, S], BF16, kind="ExternalInput")
    cfs_d = nc.dram_tensor("cfs", [1, 641], F32R, kind="ExternalInput")
    onescol_f_d = nc.dram_tensor("onescol_f", [128, 1], F32R, kind="ExternalInput")
    onescol_b_d = nc.dram_tensor("onescol_b", [128, 1], BF16, kind="ExternalInput")
    # [ident(128) | M2a(1024) | M2b(1024)]: staircase causal masks for the
    # two diagonal k-tile pairs of a 512-wide q block
    idm_d = nc.dram_tensor("idm", [128, 128 + 2048], BF16, kind="ExternalInput")
    outT_d = nc.dram_tensor("outT", [128, NDT * S], BF16, kind="ExternalOutput")

    with TileContext(nc) as tc:
        with tc.tile_pool(name="res", bufs=1) as res, \
             tc.tile_pool(name="work", bufs=2) as wk, \
             tc.tile_pool(name="pwork", bufs=2, space="PSUM") as pw:

            # ---- resident tiles (allocated once) ----
            xT = res.tile([128, NDT * S], BF16)            # [d-part, (dt, s)]
            wqT = res.tile([128, RQ * NDT * HD], BF16)     # head-major
            wkT = res.tile([128, NDT * HD], BF16)
            wvT = res.tile([128, NDT * HD], BF16)
            wpT = res.tile([128, NDT * RQ * 128], BF16)
            cosF = res.tile([128, S], BF16)
            sinF = res.tile([128, S], BF16)
            cfs = res.tile([1, 641], F32R)
            onescol_f = res.tile([128, 1], F32R)
            onescol_b = res.tile([128, 1], BF16)
            idm = res.tile([128, 128 + 2048], BF16)
            kT = res.tile([128, S], BF16)
            qT = [res.tile([128, S], BF16, name=f"qT{h}", tag=f"qT{h}")
                  for h in range(RQ)]
            yT = [res.tile([128, S], BF16, name=f"yT{h}", tag=f"yT{h}")
                  for h in range(RQ)]
            vTst = res.tile([128, S], BF16)                # v^T staging
            V_all = res.tile([128, S], BF16)               # v natural, kt-major

            eps_ap = cfs[0:1, 0:1].bitcast(F32)
            ident = idm[:, 0:128]
            M2 = [idm[:, 128:128 + 1024], idm[:, 128 + 1024:128 + 2048]]

            def body(_iv=None):
                # ---- DMA issue order: k/v weights (first matmuls), then all
                # x dt-tiles (PE wavefront), then stats consts / rope tables /
                # q weights (first needed after the last x tile anyway), then
                # attention consts and the output-projection weights.
                nc.sync.dma_start(wkT[:], wkT_d[:])
                nc.sync.dma_start(wvT[:], wvT_d[:])
                for dt in range(NDT):
                    nc.sync.dma_start(xT[:, dt * S:(dt + 1) * S],
                                      xT_d[:, dt * S:(dt + 1) * S])
                nc.sync.dma_start(cfs[:], cfs_d[:])
                nc.sync.dma_start(onescol_f[:], onescol_f_d[:])
                nc.sync.dma_start(cosF[:], cosF_d[:])
                nc.sync.dma_start(sinF[:], sinF_d[:])
                for h in range(RQ):
                    nc.sync.dma_start(
                        wqT[:, h * NDT * HD:(h + 1) * NDT * HD],
                        wqT_d[:, h * NDT * HD:(h + 1) * NDT * HD])
                nc.sync.dma_start(onescol_b[:], onescol_b_d[:])
                nc.sync.dma_start(idm[:], idm_d[:])
                nc.sync.dma_start(wpT[:], wpT_d[:])

                def proj_accum(wt_block, j, tag="big", tbufs=2):
                    """psum [128,512] = sum_d W[d].T @ xT[d, s-slice].
                    wt_block: [128, NDT*HD] slice (dt-strided weight cols)."""
                    acc = pw.tile([128, 512], F32, name="acc", tag=tag,
                                  bufs=tbufs)
                    for dt in range(NDT):
                        lhsT = wt_block[:, dt * HD:dt * HD + 128]
                        rhs = xT[:, dt * S + 512 * j: dt * S + 512 * j + 512]
                        nc.tensor.matmul(acc[:], lhsT, rhs,
                                         start=(dt == 0), stop=(dt == NDT - 1))
                    return acc

                def norm_rope_chain(acc, lnb, dest, j):
                    """RMS-norm + RoPE + scale; writes dest[:, 512j:+512] bf16.

                    rsqrt of the mean-square is exp(-0.5*ln(.)) so ScalarE
                    never leaves the natural_log_exp table set; the per-head
                    gain/sqrt(hd) factor rides the Exp bias (lnb).  The whole
                    pipeline is bf16 (2x DVE rate); stats stay fp32."""
                    stage = wk.tile([128, 512], BF16, name="stage", tag="stage", bufs=2)
                    nc.vector.tensor_copy(stage[:], acc[:])
                    swap = wk.tile([128, 512], BF16, name="swap", tag="swap", bufs=2)
                    nc.sync.dma_start(swap[0:64, :], stage[64:128, :])
                    nc.sync.dma_start(swap[64:128, :], stage[0:64, :])
                    sq = wk.tile([128, 512], F32R, name="sq", tag="sq", bufs=2)
                    nc.scalar.square(sq[:], stage[:])
                    ms = pw.tile([1, 512], F32, name="ms", tag="bcast", bufs=1)
                    nc.tensor.matmul(ms[:], onescol_f[:],
                                     sq[:], start=True, stop=True)
                    lg = wk.tile([1, 512], F32, name="lg", tag="stat", bufs=4)
                    nc.scalar.activation(lg[:], ms[:], AF.Ln,
                                         bias=eps_ap, scale=1.0 / HD)
                    rg = wk.tile([1, 512], BF16, name="rg", tag="stat", bufs=4)
                    nc.scalar.activation(rg[:], lg[:], AF.Exp,
                                         scale=-0.5, bias=lnb)
                    Rb = wk.tile([128, 512], BF16, name="Rb", tag="Rb", bufs=2)
                    nc.gpsimd.partition_broadcast(Rb[:], rg[0:1, :])
                    sl = slice(512 * j, 512 * j + 512)
                    nc.vector.tensor_mul(stage[:], stage[:], cosF[:, sl])
                    nc.vector.tensor_mul(swap[:], swap[:], sinF[:, sl])
                    nc.vector.tensor_add(stage[:], stage[:], swap[:])
                    nc.vector.tensor_mul(dest[:, sl], stage[:], Rb[:])

                def emit_proj_tail(kind, h, j, acc):
                    if kind == "k":
                        norm_rope_chain(acc, 0.0, kT, j)
                    elif kind == "q":
                        lnb = cfs[0:1, 129 + h:130 + h].bitcast(F32)
                        norm_rope_chain(acc, lnb, qT[h], j)
                    else:
                        nc.scalar.copy(vTst[:, 512 * j:512 * j + 512], acc[:])
                        # v block j is final: transpose its 4 k-tiles now so
                        # V_all[kt<=4j+3] is ready before attention block j
                        for kt in range(4 * j, 4 * j + 4):
                            tp = pw.tile([128, 128], BF16, name="tp",
                                         tag="acc", bufs=2)
                            nc.tensor.transpose(
                                tp[:], vTst[:, 128 * kt:128 * kt + 128], ident)
                            nc.scalar.copy(
                                V_all[:, 128 * kt:128 * kt + 128], tp[:])

                # ---- k/v wave: 5 chains dt-interleaved across 5 psum banks
                # so every arriving x dt-tile unlocks 5 matmuls (PE rides the
                # DMA wavefront); bcast bank stays free for the norm ms.
                kv_jobs = [("k", 0, j) for j in range(NST)] + [("v", 0, 0)]
                kv_tags = [("big", 2), ("big", 2), ("acc", 2),
                           ("acc", 2), ("small", 1)]
                kv_accs = [pw.tile([128, 512], F32, name="acc", tag=t, bufs=b)
                           for t, b in kv_tags]
                for dt in range(NDT):
                    for (kind, _h, j), acc in zip(kv_jobs, kv_accs):
                        wt = wkT if kind == "k" else wvT
                        nc.tensor.matmul(
                            acc[:], wt[:, dt * HD:dt * HD + 128],
                            xT[:, dt * S + 512 * j: dt * S + 512 * j + 512],
                            start=(dt == 0), stop=(dt == NDT - 1),
                            skip_group_check=True)
                pending = [(kind, _h, j, acc)
                           for (kind, _h, j), acc in zip(kv_jobs, kv_accs)]

                def run_proj_jobs(jobs, pending, maxlen=4):
                    for kind, h, j in jobs:
                        if kind == "q":
                            wt = wqT[:, h * NDT * HD:(h + 1) * NDT * HD]
                        else:
                            wt = wvT
                        acc = proj_accum(wt, j)
                        pending.append((kind, h, j, acc))
                        while len(pending) > maxlen:
                            emit_proj_tail(*pending.pop(0))
                    return pending

                pending = run_proj_jobs(
                    [("v", 0, j) for j in range(1, NST)], pending)

                def attention_block(h, j, fillers):
                    """Causal attention for queries [512j, 512j+512), head h.

                    k-tiles run in pairs: both score matmuls land in one
                    [128,1024] PSUM tile, one ACTIVATE exponentiates the
                    pair, DVE sums pairs (and pairs-of-pairs) so the
                    denominator matmul runs once per 4 k-tiles.  The last
                    two pairs are the causal diagonal: score matmuls run
                    full-width (so exp never sees unwritten PSUM) and the
                    precomputed masks M2 zero everything non-causal.  After
                    each consumed pair one output-projection filler closure
                    runs to keep PE fed while ACT streams exps."""
                    npair = 2 * j + 2
                    acc_y = pw.tile([128, 512], F32, name="acc_y", tag="acc",
                                    bufs=2)
                    acc_d = pw.tile([1, 512], F32, name="acc_d", tag="small",
                                    bufs=1)
                    lagged = []
                    ps_hist = {}

                    def consume(p, P2, Ps):
                        for r in (0, 1):
                            kt = 2 * p + r
                            c0 = max(0, 128 * (kt - 4 * j))
                            nc.tensor.matmul(
                                acc_y[:, c0:512],
                                V_all[:, 128 * kt:128 * kt + 128],
                                P2[:, 512 * r + c0:512 * r + 512],
                                start=(p == 0 and r == 0),
                                stop=(p == npair - 1 and r == 1),
                                skip_group_check=True)
                        if p % 2 == 1:
                            t = p // 2
                            psq = wk.tile([128, 512], BF16, name="psq",
                                          tag="psq", bufs=2)
                            nc.vector.tensor_add(psq[:], ps_hist.pop(p - 1)[:],
                                                 Ps[:])
                            nc.tensor.matmul(acc_d[:], onescol_b[:], psq[:],
                                             start=(t == 0), stop=(t == j),
                                             skip_group_check=True)
                        else:
                            ps_hist[p] = Ps
                        if fillers:
                            fillers.pop(0)()

                    for p in range(npair):
                        ps2 = pw.tile([128, 1024], F32, name="ps2", tag="big",
                                      bufs=2)
                        for r in (0, 1):
                            kt = 2 * p + r
                            # full width even on the diagonal: exp reads the
                            # whole tile, and a never-written PSUM region
                            # holds arbitrary bits -> exp could be inf and
                            # inf * mask0 = NaN.  Written scores are bounded.
                            nc.tensor.matmul(
                                ps2[:, 512 * r:512 * r + 512],
                                kT[:, 128 * kt:128 * kt + 128],
                                qT[h][:, 512 * j:512 * j + 512],
                                start=True, stop=True, skip_group_check=True)
                        P2 = wk.tile([128, 1024], BF16, name="P2", tag="P",
                                     bufs=4)
                        nc.scalar.activation(P2[:], ps2[:], AF.Exp)
                        if p >= 2 * j:                      # diagonal pair
                            nc.vector.tensor_mul(P2[:], P2[:], M2[p - 2 * j])
                        Ps = wk.tile([128, 512], BF16, name="Ps", tag="Ps",
                                     bufs=4)
                        nc.vector.tensor_add(Ps[:], P2[:, 0:512],
                                             P2[:, 512:1024])
                        lagged.append((p, P2, Ps))
                        if len(lagged) > 1:
                            consume(*lagged.pop(0))
                    while lagged:
                        consume(*lagged.pop(0))
                    rd = wk.tile([1, 512], F32, name="rd", tag="stat", bufs=4)
                    nc.vector.reciprocal_approx_fast(rd[:], acc_d[:])
                    Rd = wk.tile([128, 512], F32, name="Rd", tag="Rb", bufs=2)
                    nc.gpsimd.partition_broadcast(Rd[:], rd[0:1, :])
                    nc.vector.tensor_mul(yT[h][:, 512 * j:512 * j + 512],
                                         acc_y[:], Rd[:])

                def make_po2(dt0, j, alt):
                    """Output-projection filler: out^T partial for dt0, dt0+1
                    at q-block j -> two PSUM banks, one cast, two stores."""
                    def emit():
                        po2 = pw.tile([128, 1024], F32, name="po2", tag="big",
                                      bufs=2)
                        for half in (0, 1):
                            dt = dt0 + half
                            for hh in range(RQ):
                                nc.tensor.matmul(
                                    po2[:, 512 * half:512 * half + 512],
                                    wpT[:, dt * RQ * 128 + 128 * hh:
                                        dt * RQ * 128 + 128 * hh + 128],
                                    yT[hh][:, 512 * j:512 * j + 512],
                                    start=(hh == 0), stop=(hh == RQ - 1))
                        ost = wk.tile([128, 1024], BF16, name="ost", tag="ost",
                                      bufs=3)
                        if alt:
                            nc.scalar.copy(ost[:], po2[:])
                        else:
                            nc.vector.tensor_copy(ost[:], po2[:])
                        for half in (0, 1):
                            dt = dt0 + half
                            nc.sync.dma_start(
                                outT_d[:, dt * S + 512 * j:
                                       dt * S + 512 * j + 512],
                                ost[:, 512 * half:512 * half + 512])
                    return emit

                # ---- main loop: per q-block j, the 4 head chains, then ALL
                # their norm tails (each PSUM slot an attention op will cycle
                # onto must have its releasing tail emitted first -- engine
                # queues are FIFO, so a later-emitted release would deadlock
                # against the attention block's cross-engine waits), then the
                # 4 attention blocks.  Output-projection tiles of column j-1
                # are woven between tails and into attention consumes as PE
                # filler so PE stays busy while ACT does the norm stats / exp.
                fillers = []
                for j in range(NST):
                    pending = run_proj_jobs(
                        [("q", h, j) for h in range(RQ)], pending)
                    while pending:
                        emit_proj_tail(*pending.pop(0))
                        if fillers:
                            fillers.pop(0)()
                    for h in range(RQ):
                        attention_block(h, j, fillers)
                    fillers += [make_po2(2 * i, j, i % 2 == 0)
                                for i in range(NDT // 2)]
                for f in fillers:
                    f()

            if n_timing_iters > 1:
                with tc.For_i(0, n_timing_iters, 1):
                    body()
            else:
                body()

    # Steer the ACT-table chooser: the kernel only uses {Copy, Square, Ln,
    # Exp}, all present in natural_log_exp_and_others.  The default greedy
    # pick resolves Ln->natural_log and Exp->exp_and_others, ping-ponging
    # table loads (~2.7us each, stalls PE).  Hiding Ln/Exp from every other
    # set makes the combined set the only candidate, so it loads once.
    _orig_gat = bacc.get_activation_tables
    def _gat_one_set(arch):
        tabs = _orig_gat(arch)
        for name, fns in tabs.items():
            if name != "natural_log_exp_and_others":
                fns.discard(AF.Ln)
                fns.discard(AF.Exp)
        return tabs
    bacc.get_activation_tables = _gat_one_set
    try:
        nc.compile()
    finally:
        bacc.get_activation_tables = _orig_gat
    return nc


def _get_program(n_timing_iters=1):
    key = n_timing_iters
    if key not in _PROG_CACHE:
        _PROG_CACHE[key] = _build_program(n_timing_iters)
    return _PROG_CACHE[key]


def _host_inputs(x, Wq, Wk, Wv, Wproj, q_gain):
    """Build the 8 per-core input maps (host-side layout prep)."""
    half = HD // 2
    inv = 1.0 / (10000.0 ** (np.arange(0, HD, 2, dtype=np.float64) / HD))
    t = np.arange(S, dtype=np.float64)
    fr = np.outer(t, inv).astype(np.float32)          # [S, 64]
    cos = np.cos(fr).astype(np.float32)
    sin = np.sin(fr).astype(np.float32)
    cosF = np.concatenate([cos.T, cos.T], 0).astype(bf)          # [128, S]
    sinF = np.concatenate([sin.T, -sin.T], 0).astype(bf)

    onescol_f = np.ones((128, 1), np.float32)
    onescol_b = np.ones((128, 1), bf)
    ident = np.eye(128, dtype=np.float32)
    triu = (np.arange(128)[None, :] >= np.arange(128)[:, None]).astype(np.float32)
    ones128 = np.ones((128, 128), np.float32)
    zeros128 = np.zeros((128, 128), np.float32)

    def mrel(r):
        return np.concatenate([zeros128] * r + [triu] + [ones128] * (3 - r), 1)

    idm = np.concatenate(
        [ident, mrel(0), mrel(1), mrel(2), mrel(3)], 1).astype(bf)

    # x^T per batch: [128, (dt, s)] partition-major
    xT = [np.ascontiguousarray(
            x[b].reshape(S, NDT, 128).transpose(2, 1, 0)).reshape(128, NDT * S)
          .astype(bf)
          for b in range(B)]

    in_maps = []
    for c in range(NCORES):
        b, g = c // HKV, c % HKV
        # wq head-major: head h -> [128, (dt, c)] like k/v
        wq_heads = []
        for h in range(RQ):
            whT = Wq[512 * g + 128 * h: 512 * g + 128 * (h + 1)].T  # [D, 128]
            wq_heads.append(
                whT.reshape(NDT, 128, HD).transpose(1, 0, 2).reshape(128, NDT * HD))
        wq = np.ascontiguousarray(np.concatenate(wq_heads, axis=1)).astype(bf)
        wk_ = np.ascontiguousarray(
            Wk[128 * g:128 * (g + 1)].T.reshape(NDT, 128, HD)
            .transpose(1, 0, 2).reshape(128, NDT * HD)).astype(bf)
        wv = np.ascontiguousarray(
            Wv[128 * g:128 * (g + 1)].T.reshape(NDT, 128, HD)
            .transpose(1, 0, 2).reshape(128, NDT * HD)).astype(bf)
        wpT = np.ascontiguousarray(Wproj[:, 512 * g:512 * (g + 1)].T)  # [512, 2048]
        # [c-part 128, (dt, h, m)]: wpT2[c, dt*512+128h+m] = Wp[128dt+m, 512g+128h+c]
        wpT = np.ascontiguousarray(
            wpT.reshape(RQ, 128, NDT, 128).transpose(1, 2, 0, 3).reshape(
                128, NDT * RQ * 128)).astype(bf)
        cfsv = np.zeros((1, 641), np.float32)
        cfsv[0, 0] = EPS
        cfsv[0, 1:129] = 1.0
        gv = (q_gain[RQ * g: RQ * (g + 1)].astype(np.float64)
              / np.sqrt(HD))
        cfsv[0, 129:133] = np.log(np.maximum(gv, 1e-30)).astype(np.float32)
        in_maps.append({
            "xT": xT[b],
            "wqT": wq,
            "wkT": wk_,
            "wvT": wv,
            "wpT": wpT,
            "cosF": cosF, "sinF": sinF, "cfs": cfsv,
            "onescol_f": onescol_f, "onescol_b": onescol_b, "idm": idm,
        })
    return in_maps


def kernel(x, Wq, Wk, Wv, Wproj, q_gain, _n_timing_iters=1, _return_raw=False):
    x = np.asarray(x, np.float32)
    in_maps = _host_inputs(np.asarray(x, np.float32),
                           np.asarray(Wq, np.float32),
                           np.asarray(Wk, np.float32),
                           np.asarray(Wv, np.float32),
                           np.asarray(Wproj, np.float32),
                           np.asarray(q_gain, np.float32))
    nc = _get_program(_n_timing_iters)
    res = run_bass_kernel_spmd(nc, in_maps, core_ids=list(range(NCORES)),
                               trace=False)
    if _return_raw:
        return res
    out = np.zeros((B, S, D), np.float32)
    for c in range(NCORES):
        b = c // HKV
        outT = res.results[c]["outT"].astype(np.float32).reshape(
            128, NDT, S).transpose(1, 0, 2).reshape(D, S)
        out[b] += outT.T
    return out


if __name__ == "__main__":
    rng = np.random.default_rng(0)
    x = rng.standard_normal((B, S, D)).astype(np.float32)
    Wq = (rng.standard_normal((D, D)) * 0.02).astype(np.float32)
    Wk = (rng.standard_normal((512, D)) * 0.02).astype(np.float32)
    Wv = (rng.standard_normal((512, D)) * 0.02).astype(np.float32)
    Wp = (rng.standard_normal((D, D)) * 0.02).astype(np.float32)
    g = np.ones(H, np.float32)
    out = kernel(x, Wq, Wk, Wv, Wp, g)
    print("out", out.shape, out.dtype, float(np.abs(out).max()))


# revision 21
# speedup vs baseline: 1.1682x; 1.1682x over previous
"""Causal self-attention (GQA + RMS-norm + RoPE) Trainium2 Bass kernel.

Sharding over 8 NeuronCores: 2-way data parallel (batch) x 4-way head
parallel (one GQA group of 4 q-heads + 1 kv-head per core).  Each core
computes q/k/v projections for its group, flash-style causal attention
(scores kept transposed [k, q] so softmax sums ride the tensor engine),
and a partial output projection.  Host sums the 4 per-group partials per
batch.

All matmul operands are bf16 (fp32 accumulation in PSUM); norm / softmax
statistics are fp32.  RMS-norm bounds |scores| <= gain*sqrt(hd), so
softmax needs no max-subtraction.

Perf structure (v3):
- All DRAM tensors are [128, *] partition-major; every load is one
  full-width DMA, issued k/v-weights first, then the 16 x dt-tiles (the
  k/v projection wave is dt-interleaved across 7 PSUM banks so the PE
  rides the x DMA wavefront), then everything else.
- ScalarE uses only {Copy, Square, Ln, Exp} from one table set
  (natural_log_exp_and_others): rsqrt = exp(-0.5*ln(.)).  No table
  reloads -> no PE cold windows.
- Attention scores for two adjacent k-tiles land bf16 in one PSUM bank
  and are exponentiated by a single ACTIVATE (halves ACT instruction
  overhead, the attention-phase bottleneck); causal masking uses
  precomputed staircase masks; the softmax denominator matmul runs on
  the DVE pair-sum (halves its PE cost).
- q-projections of head h+1 are emitted ahead of attention(h) so PE
  keeps dense work while ACT chews the exp stream.
- Output partials stored bf16; PSUM->SBUF casts alternate DVE/ACT.
"""

import numpy as np
import ml_dtypes

import concourse.bacc as bacc
import concourse.mybir as mybir
from concourse.tile import TileContext
from concourse.bass_utils import run_bass_kernel_spmd

BF16 = mybir.dt.bfloat16
F32 = mybir.dt.float32
F32R = mybir.dt.float32r
AF = mybir.ActivationFunctionType
bf = ml_dtypes.bfloat16

B, S, D = 2, 2048, 2048
H, HKV, HD = 16, 4, 128
RQ = H // HKV            # q heads per kv group (4)
NCORES = 8
NDT = D // 128           # 16 contraction tiles
NST = S // 512           # 4 query/sequence 512-tiles
NKT = S // 128           # 16 key 128-tiles
EPS = float(np.finfo(np.float32).eps)

_PROG_CACHE = {}


def _build_program(n_timing_iters=1):
    nc = bacc.Bacc("TRN2", debug=False, enable_asserts=False, num_devices=NCORES)

    xT_d = nc.dram_tensor("xT", [128, NDT * S], BF16, kind="ExternalInput")
    # head-major: head h occupies cols [h*NDT*HD, (h+1)*NDT*HD), dt-strided
    wqT_d = nc.dram_tensor("wqT", [128, RQ * NDT * HD], BF16, kind="ExternalInput")
    wkT_d = nc.dram_tensor("wkT", [128, NDT * HD], BF16, kind="ExternalInput")
    wvT_d = nc.dram_tensor("wvT", [128, NDT * HD], BF16, kind="ExternalInput")
    # dt-major: dt block cols [dt*RQ*128, (dt+1)*RQ*128), h within
    wpT_d = nc.dram_tensor("wpT", [128, NDT * RQ * 128], BF16, kind="ExternalInput")
    cosF_d = nc.dram_tensor("cosF", [128, S], BF16, kind="ExternalInput")
    sinF_d = nc.dram_tensor("sinF", [128, S], BF16, kind="ExternalInput")
    cfs_d = nc.dram_tensor("cfs", [1, 641], F32R, kind="ExternalInput")
    onescol_f_d = nc.dram_tensor("onescol_f", [128, 1], F32R, kind="ExternalInput")
    onescol_b_d = nc.dram_tensor("onescol_b", [128, 1], BF16, kind="ExternalInput")
    # [ident(128) | M2a(1024) | M2b(1024)]: staircase causal masks for the
    # two diagonal k-tile pairs of a 512-wide q block
    idm_d = nc.dram_tensor("idm", [128, 128 + 2048], BF16, kind="ExternalInput")
    outT_d = nc.dram_tensor("outT", [128, NDT * S], BF16, kind="ExternalOutput")

    with TileContext(nc) as tc:
        with tc.tile_pool(name="res", bufs=1) as res, \
             tc.tile_pool(name="work", bufs=2) as wk, \
             tc.tile_pool(name="pwork", bufs=2, space="PSUM") as pw:

            # ---- resident tiles (allocated once) ----
            xT = res.tile([128, NDT * S], BF16)            # [d-part, (dt, s)]
            wqT = res.tile([128, RQ * NDT * HD], BF16)     # head-major
            wkT = res.tile([128, NDT * HD], BF16)
            wvT = res.tile([128, NDT * HD], BF16)
            wpT = res.tile([128, NDT * RQ * 128], BF16)
            cosF = res.tile([128, S], BF16)
            sinF = res.tile([128, S], BF16)
            cfs = res.tile([1, 641], F32R)
            onescol_f = res.tile([128, 1], F32R)
            onescol_b = res.tile([128, 1], BF16)
            idm = res.tile([128, 128 + 2048], BF16)
            kT = res.tile([128, S], BF16)
            qT = [res.tile([128, S], BF16, name=f"qT{h}", tag=f"qT{h}")
                  for h in range(RQ)]
            yT = [res.tile([128, S], BF16, name=f"yT{h}", tag=f"yT{h}")
                  for h in range(RQ)]
            vTst = res.tile([128, S], BF16)                # v^T staging
            V_all = res.tile([128, S], BF16)               # v natural, kt-major

            eps_ap = cfs[0:1, 0:1].bitcast(F32)
            ident = idm[:, 0:128]
            M2 = [idm[:, 128:128 + 1024], idm[:, 128 + 1024:128 + 2048]]

            def body(_iv=None):
                # ---- DMA issue order: k/v weights (first matmuls), then all
                # x dt-tiles (PE wavefront), then stats consts / rope tables /
                # q weights (first needed after the last x tile anyway), then
                # attention consts and the output-projection weights.
                nc.sync.dma_start(wkT[:], wkT_d[:])
                nc.sync.dma_start(wvT[:], wvT_d[:])
                for dt in range(NDT):
                    nc.sync.dma_start(xT[:, dt * S:(dt + 1) * S],
                                      xT_d[:, dt * S:(dt + 1) * S])
                nc.sync.dma_start(cfs[:], cfs_d[:])
                nc.sync.dma_start(onescol_f[:], onescol_f_d[:])
                nc.sync.dma_start(cosF[:], cosF_d[:])
                nc.sync.dma_start(sinF[:], sinF_d[:])
                for h in range(RQ):
                    nc.sync.dma_start(
                        wqT[:, h * NDT * HD:(h + 1) * NDT * HD],
                        wqT_d[:, h * NDT * HD:(h + 1) * NDT * HD])
                nc.sync.dma_start(onescol_b[:], onescol_b_d[:])
                nc.sync.dma_start(idm[:], idm_d[:])
                nc.sync.dma_start(wpT[:], wpT_d[:])

                def proj_accum(wt_block, j, tag="big", tbufs=2):
                    """psum [128,512] = sum_d W[d].T @ xT[d, s-slice].
                    wt_block: [128, NDT*HD] slice (dt-strided weight cols)."""
                    acc = pw.tile([128, 512], F32, name="acc", tag=tag,
                                  bufs=tbufs)
                    for dt in range(NDT):
                        lhsT = wt_block[:, dt * HD:dt * HD + 128]
                        rhs = xT[:, dt * S + 512 * j: dt * S + 512 * j + 512]
                        nc.tensor.matmul(acc[:], lhsT, rhs,
                                         start=(dt == 0), stop=(dt == NDT - 1))
                    return acc

                def norm_rope_chain(acc, lnb, dest, j):
                    """RMS-norm + RoPE + scale; writes dest[:, 512j:+512] bf16.

                    rsqrt of the mean-square is exp(-0.5*ln(.)) so ScalarE
                    never leaves the natural_log_exp table set; the per-head
                    gain/sqrt(hd) factor rides the Exp bias (lnb)."""
                    stage = wk.tile([128, 512], BF16, name="stage", tag="stage", bufs=2)
                    nc.vector.tensor_copy(stage[:], acc[:])
                    swap = wk.tile([128, 512], BF16, name="swap", tag="swap", bufs=2)
                    nc.sync.dma_start(swap[0:64, :], stage[64:128, :])
                    nc.sync.dma_start(swap[64:128, :], stage[0:64, :])
                    sq = wk.tile([128, 512], F32R, name="sq", tag="sq", bufs=2)
                    nc.scalar.square(sq[:], stage[:])
                    ms = pw.tile([1, 512], F32, name="ms", tag="bcast", bufs=1)
                    nc.tensor.matmul(ms[:], onescol_f[:],
                                     sq[:], start=True, stop=True)
                    lg = wk.tile([1, 512], F32, name="lg", tag="stat", bufs=4)
                    nc.scalar.activation(lg[:], ms[:], AF.Ln,
                                         bias=eps_ap, scale=1.0 / HD)
                    rg = wk.tile([1, 512], BF16, name="rg", tag="stat", bufs=4)
                    nc.scalar.activation(rg[:], lg[:], AF.Exp,
                                         scale=-0.5, bias=lnb)
                    Rb = wk.tile([128, 512], BF16, name="Rb", tag="Rb", bufs=2)
                    nc.gpsimd.partition_broadcast(Rb[:], rg[0:1, :])
                    sl = slice(512 * j, 512 * j + 512)
                    nc.vector.tensor_mul(stage[:], stage[:], cosF[:, sl])
                    nc.vector.tensor_mul(swap[:], swap[:], sinF[:, sl])
                    nc.vector.tensor_add(stage[:], stage[:], swap[:])
                    nc.vector.tensor_mul(dest[:, sl], stage[:], Rb[:])

                def emit_proj_tail(kind, h, j, acc):
                    if kind == "k":
                        norm_rope_chain(acc, 0.0, kT, j)
                    elif kind == "q":
                        lnb = cfs[0:1, 129 + h:130 + h].bitcast(F32)
                        norm_rope_chain(acc, lnb, qT[h], j)
                    else:
                        nc.scalar.copy(vTst[:, 512 * j:512 * j + 512], acc[:])
                        # v block j is final: transpose its 4 k-tiles now, as
                        # PE filler spread through the projection phase
                        for kt in range(4 * j, 4 * j + 4):
                            tp = pw.tile([128, 128], BF16, name="tp",
                                         tag="acc", bufs=2)
                            nc.tensor.transpose(
                                tp[:], vTst[:, 128 * kt:128 * kt + 128], ident)
                            nc.scalar.copy(
                                V_all[:, 128 * kt:128 * kt + 128], tp[:])

                # ---- k/v wave: 5 chains dt-interleaved across 5 psum banks
                # so every arriving x dt-tile unlocks 5 matmuls (PE rides the
                # DMA wavefront); bcast bank stays free for the norm ms.
                kv_jobs = [("k", 0, j) for j in range(NST)] + [("v", 0, 0)]
                kv_tags = [("big", 2), ("big", 2), ("acc", 2),
                           ("acc", 2), ("small", 1)]
                kv_accs = [pw.tile([128, 512], F32, name="acc", tag=t, bufs=b)
                           for t, b in kv_tags]
                for dt in range(NDT):
                    for (kind, _h, j), acc in zip(kv_jobs, kv_accs):
                        wt = wkT if kind == "k" else wvT
                        nc.tensor.matmul(
                            acc[:], wt[:, dt * HD:dt * HD + 128],
                            xT[:, dt * S + 512 * j: dt * S + 512 * j + 512],
                            start=(dt == 0), stop=(dt == NDT - 1),
                            skip_group_check=True)
                pending = [(kind, _h, j, acc)
                           for (kind, _h, j), acc in zip(kv_jobs, kv_accs)]

                def run_proj_jobs(jobs, pending, maxlen=4):
                    for kind, h, j in jobs:
                        if kind == "q":
                            wt = wqT[:, h * NDT * HD:(h + 1) * NDT * HD]
                        else:
                            wt = wvT
                        acc = proj_accum(wt, j)
                        pending.append((kind, h, j, acc))
                        while len(pending) > maxlen:
                            emit_proj_tail(*pending.pop(0))
                    return pending

                pending = run_proj_jobs(
                    [("v", 0, j) for j in range(1, NST)], pending)

                def attention_block(h, j):
                    """Causal attention for queries [512j, 512j+512), head h.

                    k-tiles are processed in pairs: both score matmuls land
                    bf16 in one PSUM bank, one ACTIVATE exponentiates the
                    [128,1024] pair, the DVE pair-sum feeds a single
                    denominator matmul.  The last two pairs are the causal
                    diagonal; they run full-width and are zeroed/triangled by
                    the precomputed staircase masks M2[0], M2[1]."""
                    npair = 2 * j + 2
                    acc_y = pw.tile([128, 512], F32, name="acc_y", tag="acc",
                                    bufs=2)
                    acc_d = pw.tile([1, 512], F32, name="acc_d", tag="small",
                                    bufs=1)
                    lagged = []
                    ps_hist = {}

                    def consume(p, P2, Ps):
                        for r in (0, 1):
                            kt = 2 * p + r
                            c0 = max(0, 128 * (kt - 4 * j))
                            nc.tensor.matmul(
                                acc_y[:, c0:512],
                                V_all[:, 128 * kt:128 * kt + 128],
                                P2[:, 512 * r + c0:512 * r + 512],
                                start=(p == 0 and r == 0),
                                stop=(p == npair - 1 and r == 1),
                                skip_group_check=True)
                        # denominator matmul once per 4 k-tiles on the
                        # DVE-summed P (quarters its PE cost)
                        if p % 2 == 1:
                            t = p // 2
                            psq = wk.tile([128, 512], BF16, name="psq",
                                          tag="psq", bufs=2)
                            nc.vector.tensor_add(psq[:], ps_hist.pop(p - 1)[:],
                                                 Ps[:])
                            nc.tensor.matmul(acc_d[:], onescol_b[:], psq[:],
                                             start=(t == 0), stop=(t == j),
                                             skip_group_check=True)
                        else:
                            ps_hist[p] = Ps

                    for p in range(npair):
                        ps2 = pw.tile([128, 1024], F32, name="ps2", tag="big",
                                      bufs=2)
                        for r in (0, 1):
                            kt = 2 * p + r
                            nc.tensor.matmul(
                                ps2[:, 512 * r:512 * r + 512],
                                kT[:, 128 * kt:128 * kt + 128],
                                qT[h][:, 512 * j:512 * j + 512],
                                start=True, stop=True, skip_group_check=True)
                        P2 = wk.tile([128, 1024], BF16, name="P2", tag="P",
                                     bufs=4)
                        nc.scalar.activation(P2[:], ps2[:], AF.Exp)
                        if p >= 2 * j:                      # diagonal pair
                            nc.vector.tensor_mul(P2[:], P2[:], M2[p - 2 * j])
                        Ps = wk.tile([128, 512], BF16, name="Ps", tag="Ps",
                                     bufs=4)
                        nc.vector.tensor_add(Ps[:], P2[:, 0:512],
                                             P2[:, 512:1024])
                        lagged.append((p, P2, Ps))
                        if len(lagged) > 1:
                            consume(*lagged.pop(0))
                    while lagged:
                        consume(*lagged.pop(0))
                    rd = wk.tile([1, 512], F32, name="rd", tag="stat", bufs=4)
                    nc.vector.reciprocal_approx_fast(rd[:], acc_d[:])
                    Rd = wk.tile([128, 512], F32, name="Rd", tag="Rb", bufs=2)
                    nc.gpsimd.partition_broadcast(Rd[:], rd[0:1, :])
                    nc.vector.tensor_mul(yT[h][:, 512 * j:512 * j + 512],
                                         acc_y[:], Rd[:])

                # ---- per q-head: q(h+1) projections emitted ahead of
                # attention(h); head h's LAST (largest) attention block is
                # deferred until after the h+1 norm tails, so its score
                # matmuls keep PE busy while ACT chews the bunched norm
                # stats.  (Tails must precede any attention block whose PSUM
                # slots they release -- engine FIFOs would deadlock
                # otherwise -- so the bunch itself can't be split up.)
                pending = run_proj_jobs([("q", 0, j) for j in range(NST)],
                                        pending)
                while pending:
                    emit_proj_tail(*pending.pop(0))
                for h in range(RQ):
                    for j in range(NST - 1):
                        attention_block(h, j)
                    if h + 1 < RQ:
                        pending = run_proj_jobs(
                            [("q", h + 1, j) for j in range(NST)], pending)
                        while pending:
                            emit_proj_tail(*pending.pop(0))
                    attention_block(h, NST - 1)

                # ---- output projection (transposed: out^T[D, s], bf16);
                # PSUM->SBUF casts alternate DVE/ACT to split the load ----
                ptags = ["big", "acc", "bcast", "small"]
                pbufs = {"big": 2, "acc": 2, "bcast": 1, "small": 1}
                for dt in range(NDT):
                    osb = wk.tile([128, S], BF16, name="osb", tag="osb")
                    for sjj in range(NST):
                        po = pw.tile([128, 512], F32, name=f"po{sjj}",
                                     tag=ptags[sjj], bufs=pbufs[ptags[sjj]])
                        for h in range(RQ):
                            nc.tensor.matmul(
                                po[:],
                                wpT[:, dt * RQ * 128 + 128 * h:
                                    dt * RQ * 128 + 128 * h + 128],
                                yT[h][:, 512 * sjj:512 * sjj + 512],
                                start=(h == 0), stop=(h == RQ - 1))
                        dst = osb[:, 512 * sjj:512 * sjj + 512]
                        if sjj % 2 == 0:
                            nc.vector.tensor_copy(dst, po[:])
                        else:
                            nc.scalar.copy(dst, po[:])
                    nc.sync.dma_start(outT_d[:, dt * S:(dt + 1) * S], osb[:])

            if n_timing_iters > 1:
                with tc.For_i(0, n_timing_iters, 1):
                    body()
            else:
                body()

    # Steer the ACT-table chooser: the kernel only uses {Copy, Square, Ln,
    # Exp}, all present in natural_log_exp_and_others.  The default greedy
    # pick resolves Ln->natural_log and Exp->exp_and_others, ping-ponging
    # table loads (~2.7us each, stalls PE).  Hiding Ln/Exp from every other
    # set makes the combined set the only candidate, so it loads once.
    _orig_gat = bacc.get_activation_tables
    def _gat_one_set(arch):
        tabs = _orig_gat(arch)
        for name, fns in tabs.items():
            if name != "natural_log_exp_and_others":
                fns.discard(AF.Ln)
                fns.discard(AF.Exp)
        return tabs
    bacc.get_activation_tables = _gat_one_set
    try:
        nc.compile()
    finally:
        bacc.get_activation_tables = _orig_gat
    return nc


def _get_program(n_timing_iters=1):
    key = n_timing_iters
    if key not in _PROG_CACHE:
        _PROG_CACHE[key] = _build_program(n_timing_iters)
    return _PROG_CACHE[key]


def _host_inputs(x, Wq, Wk, Wv, Wproj, q_gain):
    """Build the 8 per-core input maps (host-side layout prep)."""
    half = HD // 2
    inv = 1.0 / (10000.0 ** (np.arange(0, HD, 2, dtype=np.float64) / HD))
    t = np.arange(S, dtype=np.float64)
    fr = np.outer(t, inv).astype(np.float32)          # [S, 64]
    cos = np.cos(fr).astype(np.float32)
    sin = np.sin(fr).astype(np.float32)
    cosF = np.concatenate([cos.T, cos.T], 0).astype(bf)          # [128, S]
    sinF = np.concatenate([sin.T, -sin.T], 0).astype(bf)

    onescol_f = np.ones((128, 1), np.float32)
    onescol_b = np.ones((128, 1), bf)
    ident = np.eye(128, dtype=np.float32)
    triu = (np.arange(128)[None, :] >= np.arange(128)[:, None]).astype(np.float32)
    ones128 = np.ones((128, 128), np.float32)
    zeros128 = np.zeros((128, 128), np.float32)

    def mrel(r):
        return np.concatenate([zeros128] * r + [triu] + [ones128] * (3 - r), 1)

    idm = np.concatenate(
        [ident, mrel(0)[:, 0:512], mrel(1)[:, 0:512],
         mrel(2)[:, 0:512], mrel(3)[:, 0:512]], 1).astype(bf)

    # x^T per batch: [128, (dt, s)] partition-major
    xT = [np.ascontiguousarray(
            x[b].reshape(S, NDT, 128).transpose(2, 1, 0)).reshape(128, NDT * S)
          .astype(bf)
          for b in range(B)]

    in_maps = []
    for c in range(NCORES):
        b, g = c // HKV, c % HKV
        # wq head-major: head h -> [128, (dt, c)] like k/v
        wq_heads = []
        for h in range(RQ):
            whT = Wq[512 * g + 128 * h: 512 * g + 128 * (h + 1)].T  # [D, 128]
            wq_heads.append(
                whT.reshape(NDT, 128, HD).transpose(1, 0, 2).reshape(128, NDT * HD))
        wq = np.ascontiguousarray(np.concatenate(wq_heads, axis=1)).astype(bf)
        wk_ = np.ascontiguousarray(
            Wk[128 * g:128 * (g + 1)].T.reshape(NDT, 128, HD)
            .transpose(1, 0, 2).reshape(128, NDT * HD)).astype(bf)
        wv = np.ascontiguousarray(
            Wv[128 * g:128 * (g + 1)].T.reshape(NDT, 128, HD)
            .transpose(1, 0, 2).reshape(128, NDT * HD)).astype(bf)
        wpT = np.ascontiguousarray(Wproj[:, 512 * g:512 * (g + 1)].T)  # [512, 2048]
        # [c-part 128, (dt, h, m)]: wpT2[c, dt*512+128h+m] = Wp[128dt+m, 512g+128h+c]
        wpT = np.ascontiguousarray(
            wpT.reshape(RQ, 128, NDT, 128).transpose(1, 2, 0, 3).reshape(
                128, NDT * RQ * 128)).astype(bf)
        cfsv = np.zeros((1, 641), np.float32)
        cfsv[0, 0] = EPS
        cfsv[0, 1:129] = 1.0
        gv = (q_gain[RQ * g: RQ * (g + 1)].astype(np.float64)
              / np.sqrt(HD))
        cfsv[0, 129:133] = np.log(np.maximum(gv, 1e-30)).astype(np.float32)
        in_maps.append({
            "xT": xT[b],
            "wqT": wq,
            "wkT": wk_,
            "wvT": wv,
            "wpT": wpT,
            "cosF": cosF, "sinF": sinF, "cfs": cfsv,
            "onescol_f": onescol_f, "onescol_b": onescol_b, "idm": idm,
        })
    return in_maps


def kernel(x, Wq, Wk, Wv, Wproj, q_gain, _n_timing_iters=1, _return_raw=False):
    x = np.asarray(x, np.float32)
    in_maps = _host_inputs(np.asarray(x, np.float32),
                           np.asarray(Wq, np.float32),
                           np.asarray(Wk, np.float32),
                           np.asarray(Wv, np.float32),
                           np.asarray(Wproj, np.float32),
                           np.asarray(q_gain, np.float32))
    nc = _get_program(_n_timing_iters)
    res = run_bass_kernel_spmd(nc, in_maps, core_ids=list(range(NCORES)),
                               trace=False)
    if _return_raw:
        return res
    out = np.zeros((B, S, D), np.float32)
    for c in range(NCORES):
        b = c // HKV
        outT = res.results[c]["outT"].astype(np.float32).reshape(
            128, NDT, S).transpose(1, 0, 2).reshape(D, S)
        out[b] += outT.T
    return out


if __name__ == "__main__":
    rng = np.random.default_rng(0)
    x = rng.standard_normal((B, S, D)).astype(np.float32)
    Wq = (rng.standard_normal((D, D)) * 0.02).astype(np.float32)
    Wk = (rng.standard_normal((512, D)) * 0.02).astype(np.float32)
    Wv = (rng.standard_normal((512, D)) * 0.02).astype(np.float32)
    Wp = (rng.standard_normal((D, D)) * 0.02).astype(np.float32)
    g = np.ones(H, np.float32)
    out = kernel(x, Wq, Wk, Wv, Wp, g)
    print("out", out.shape, out.dtype, float(np.abs(out).max()))


# revision 26
# speedup vs baseline: 1.2712x; 1.0882x over previous
"""Causal self-attention (GQA + RMS-norm + RoPE) Trainium2 Bass kernel.

Sharding over 8 NeuronCores: 2-way data parallel (batch) x 4-way head
parallel (one GQA group of 4 q-heads + 1 kv-head per core).  Each core
computes q/k/v projections for its group, flash-style causal attention
(scores kept transposed [k, q] so softmax sums ride the tensor engine),
and a partial output projection.  Host sums the 4 per-group partials per
batch.

All matmul operands are bf16 (fp32 accumulation in PSUM); norm / softmax
statistics are fp32.  RMS-norm bounds |scores| <= gain*sqrt(hd), so
softmax needs no max-subtraction.

Perf structure (v3):
- All DRAM tensors are [128, *] partition-major; every load is one
  full-width DMA, issued k/v-weights first, then the 16 x dt-tiles (the
  k/v projection wave is dt-interleaved across 7 PSUM banks so the PE
  rides the x DMA wavefront), then everything else.
- ScalarE uses only {Copy, Square, Ln, Exp} from one table set
  (natural_log_exp_and_others): rsqrt = exp(-0.5*ln(.)).  No table
  reloads -> no PE cold windows.
- Attention scores for two adjacent k-tiles land bf16 in one PSUM bank
  and are exponentiated by a single ACTIVATE (halves ACT instruction
  overhead, the attention-phase bottleneck); causal masking uses
  precomputed staircase masks; the softmax denominator matmul runs on
  the DVE pair-sum (halves its PE cost).
- q-projections of head h+1 are emitted ahead of attention(h) so PE
  keeps dense work while ACT chews the exp stream.
- Output partials stored bf16; PSUM->SBUF casts alternate DVE/ACT.
"""

import numpy as np
import ml_dtypes

import concourse.bacc as bacc
import concourse.mybir as mybir
from concourse.tile import TileContext
from concourse.bass_utils import run_bass_kernel_spmd

BF16 = mybir.dt.bfloat16
F32 = mybir.dt.float32
F32R = mybir.dt.float32r
AF = mybir.ActivationFunctionType
bf = ml_dtypes.bfloat16

B, S, D = 2, 2048, 2048
H, HKV, HD = 16, 4, 128
RQ = H // HKV            # q heads per kv group (4)
NCORES = 8
NDT = D // 128           # 16 contraction tiles
NST = S // 512           # 4 query/sequence 512-tiles
NKT = S // 128           # 16 key 128-tiles
EPS = float(np.finfo(np.float32).eps)

_PROG_CACHE = {}


def _build_program(n_timing_iters=1):
    nc = bacc.Bacc("TRN2", debug=False, enable_asserts=False, num_devices=NCORES)

    xT_d = nc.dram_tensor("xT", [128, NDT * S], BF16, kind="ExternalInput")
    # head-major: head h occupies cols [h*NDT*HD, (h+1)*NDT*HD), dt-strided
    wqT_d = nc.dram_tensor("wqT", [128, RQ * NDT * HD], BF16, kind="ExternalInput")
    wkT_d = nc.dram_tensor("wkT", [128, NDT * HD], BF16, kind="ExternalInput")
    wvT_d = nc.dram_tensor("wvT", [128, NDT * HD], BF16, kind="ExternalInput")
    # dt-major: dt block cols [dt*RQ*128, (dt+1)*RQ*128), h within
    wpT_d = nc.dram_tensor("wpT", [128, NDT * RQ * 128], BF16, kind="ExternalInput")
    cosF_d = nc.dram_tensor("cosF", [128, S], BF16, kind="ExternalInput")
    sinF_d = nc.dram_tensor("sinF", [128, S], BF16, kind="ExternalInput")
    cfs_d = nc.dram_tensor("cfs", [1, 641], F32R, kind="ExternalInput")
    onescol_f_d = nc.dram_tensor("onescol_f", [128, 1], F32R, kind="ExternalInput")
    onescol_b_d = nc.dram_tensor("onescol_b", [128, 1], BF16, kind="ExternalInput")
    # [ident(128) | M2a(1024) | M2b(1024)]: staircase causal masks for the
    # two diagonal k-tile pairs of a 512-wide q block
    idm_d = nc.dram_tensor("idm", [128, 128 + 2048], BF16, kind="ExternalInput")
    outT_d = nc.dram_tensor("outT", [128, NDT * S], BF16, kind="ExternalOutput")

    with TileContext(nc) as tc:
        with tc.tile_pool(name="res", bufs=1) as res, \
             tc.tile_pool(name="work", bufs=2) as wk, \
             tc.tile_pool(name="pwork", bufs=2, space="PSUM") as pw:

            # ---- resident tiles (allocated once) ----
            xT = res.tile([128, NDT * S], BF16)            # [d-part, (dt, s)]
            wqT = res.tile([128, RQ * NDT * HD], BF16)     # head-major
            wkT = res.tile([128, NDT * HD], BF16)
            wvT = res.tile([128, NDT * HD], BF16)
            wpT = res.tile([128, NDT * RQ * 128], BF16)
            cosF = res.tile([128, S], BF16)
            sinF = res.tile([128, S], BF16)
            cfs = res.tile([1, 641], F32R)
            onescol_f = res.tile([128, 1], F32R)
            onescol_b = res.tile([128, 1], BF16)
            idm = res.tile([128, 128 + 2048], BF16)
            kT = res.tile([128, S], BF16)
            qT = [res.tile([128, S], BF16, name=f"qT{h}", tag=f"qT{h}")
                  for h in range(RQ)]
            yT = [res.tile([128, S], BF16, name=f"yT{h}", tag=f"yT{h}")
                  for h in range(RQ)]
            vTst = res.tile([128, S], BF16)                # v^T staging
            V_all = res.tile([128, S], BF16)               # v natural, kt-major

            eps_ap = cfs[0:1, 0:1].bitcast(F32)
            ident = idm[:, 0:128]
            M2 = [idm[:, 128:128 + 1024], idm[:, 128 + 1024:128 + 2048]]

            def body(_iv=None):
                # ---- DMA issue order: k/v weights (first matmuls), then all
                # x dt-tiles (PE wavefront), then stats consts / rope tables /
                # q weights (first needed after the last x tile anyway), then
                # attention consts and the output-projection weights.
                nc.sync.dma_start(wkT[:], wkT_d[:])
                nc.sync.dma_start(wvT[:], wvT_d[:])
                for dt in range(NDT):
                    nc.sync.dma_start(xT[:, dt * S:(dt + 1) * S],
                                      xT_d[:, dt * S:(dt + 1) * S])
                nc.sync.dma_start(cfs[:], cfs_d[:])
                nc.sync.dma_start(onescol_f[:], onescol_f_d[:])
                nc.sync.dma_start(cosF[:], cosF_d[:])
                nc.sync.dma_start(sinF[:], sinF_d[:])
                for h in range(RQ):
                    nc.sync.dma_start(
                        wqT[:, h * NDT * HD:(h + 1) * NDT * HD],
                        wqT_d[:, h * NDT * HD:(h + 1) * NDT * HD])
                nc.sync.dma_start(onescol_b[:], onescol_b_d[:])
                nc.sync.dma_start(idm[:], idm_d[:])
                nc.sync.dma_start(wpT[:], wpT_d[:])

                def proj_accum(wt_block, j, tag="big", tbufs=2):
                    """psum [128,512] = sum_d W[d].T @ xT[d, s-slice].
                    wt_block: [128, NDT*HD] slice (dt-strided weight cols)."""
                    acc = pw.tile([128, 512], F32, name="acc", tag=tag,
                                  bufs=tbufs)
                    for dt in range(NDT):
                        lhsT = wt_block[:, dt * HD:dt * HD + 128]
                        rhs = xT[:, dt * S + 512 * j: dt * S + 512 * j + 512]
                        nc.tensor.matmul(acc[:], lhsT, rhs,
                                         start=(dt == 0), stop=(dt == NDT - 1))
                    return acc

                def norm_rope_chain(acc, lnb, dest, j):
                    """RMS-norm + RoPE + scale; writes dest[:, 512j:+512] bf16.

                    rsqrt of the mean-square is exp(-0.5*ln(.)) so ScalarE
                    never leaves the natural_log_exp table set; the per-head
                    gain/sqrt(hd) factor rides the Exp bias (lnb)."""
                    stage = wk.tile([128, 512], BF16, name="stage", tag="stage", bufs=2)
                    nc.vector.tensor_copy(stage[:], acc[:])
                    swap = wk.tile([128, 512], BF16, name="swap", tag="swap", bufs=2)
                    nc.sync.dma_start(swap[0:64, :], stage[64:128, :])
                    nc.sync.dma_start(swap[64:128, :], stage[0:64, :])
                    sq = wk.tile([128, 512], F32R, name="sq", tag="sq", bufs=2)
                    nc.scalar.square(sq[:], stage[:])
                    ms = pw.tile([1, 512], F32, name="ms", tag="bcast", bufs=1)
                    nc.tensor.matmul(ms[:], onescol_f[:],
                                     sq[:], start=True, stop=True)
                    lg = wk.tile([1, 512], F32, name="lg", tag="stat", bufs=4)
                    nc.scalar.activation(lg[:], ms[:], AF.Ln,
                                         bias=eps_ap, scale=1.0 / HD)
                    rg = wk.tile([1, 512], BF16, name="rg", tag="stat", bufs=4)
                    nc.scalar.activation(rg[:], lg[:], AF.Exp,
                                         scale=-0.5, bias=lnb)
                    Rb = wk.tile([128, 512], BF16, name="Rb", tag="Rb", bufs=2)
                    nc.gpsimd.partition_broadcast(Rb[:], rg[0:1, :])
                    sl = slice(512 * j, 512 * j + 512)
                    nc.vector.tensor_mul(stage[:], stage[:], cosF[:, sl])
                    nc.vector.tensor_mul(swap[:], swap[:], sinF[:, sl])
                    nc.vector.tensor_add(stage[:], stage[:], swap[:])
                    nc.vector.tensor_mul(dest[:, sl], stage[:], Rb[:])

                def emit_proj_tail(kind, h, j, acc):
                    if kind == "k":
                        norm_rope_chain(acc, 0.0, kT, j)
                    elif kind == "q":
                        lnb = cfs[0:1, 129 + h:130 + h].bitcast(F32)
                        norm_rope_chain(acc, lnb, qT[h], j)
                    else:
                        nc.scalar.copy(vTst[:, 512 * j:512 * j + 512], acc[:])
                        # v block j is final: transpose its 4 k-tiles now, as
                        # PE filler spread through the projection phase
                        for kt in range(4 * j, 4 * j + 4):
                            tp = pw.tile([128, 128], BF16, name="tp",
                                         tag="acc", bufs=2)
                            nc.tensor.transpose(
                                tp[:], vTst[:, 128 * kt:128 * kt + 128], ident)
                            nc.scalar.copy(
                                V_all[:, 128 * kt:128 * kt + 128], tp[:])

                # ---- k/v wave: 5 chains dt-interleaved across 5 psum banks
                # so every arriving x dt-tile unlocks 5 matmuls (PE rides the
                # DMA wavefront); bcast bank stays free for the norm ms.
                kv_jobs = [("k", 0, j) for j in range(NST)] + [("v", 0, 0)]
                kv_tags = [("big", 2), ("big", 2), ("acc", 2),
                           ("acc", 2), ("small", 1)]
                kv_accs = [pw.tile([128, 512], F32, name="acc", tag=t, bufs=b)
                           for t, b in kv_tags]
                for dt in range(NDT):
                    for (kind, _h, j), acc in zip(kv_jobs, kv_accs):
                        wt = wkT if kind == "k" else wvT
                        nc.tensor.matmul(
                            acc[:], wt[:, dt * HD:dt * HD + 128],
                            xT[:, dt * S + 512 * j: dt * S + 512 * j + 512],
                            start=(dt == 0), stop=(dt == NDT - 1),
                            skip_group_check=True)
                pending = [(kind, _h, j, acc)
                           for (kind, _h, j), acc in zip(kv_jobs, kv_accs)]

                def run_proj_jobs(jobs, pending, maxlen=1):
                    for kind, h, j in jobs:
                        if kind == "q":
                            wt = wqT[:, h * NDT * HD:(h + 1) * NDT * HD]
                        else:
                            wt = wvT
                        acc = proj_accum(wt, j)
                        pending.append((kind, h, j, acc))
                        while len(pending) > maxlen:
                            emit_proj_tail(*pending.pop(0))
                    return pending

                # maxlen=4 spreads the 5 k/v-wave tails across the three
                # v-chains instead of bunching them (tails have little PE
                # work, so a bunch leaves PE idle long enough to re-throttle)
                pending = run_proj_jobs(
                    [("v", 0, j) for j in range(1, NST)], pending, maxlen=4)

                def attention_block(h, j):
                    """Causal attention for queries [512j, 512j+512), head h.

                    k-tiles are processed in pairs: both score matmuls land
                    bf16 in one PSUM bank, one ACTIVATE exponentiates the
                    [128,1024] pair, the DVE pair-sum feeds a single
                    denominator matmul.  The last two pairs are the causal
                    diagonal; they run full-width and are zeroed/triangled by
                    the precomputed staircase masks M2[0], M2[1]."""
                    npair = 2 * j + 2
                    acc_y = pw.tile([128, 512], F32, name="acc_y", tag="acc",
                                    bufs=2)
                    acc_d = pw.tile([1, 512], F32, name="acc_d", tag="small",
                                    bufs=1)
                    lagged = []
                    ps_hist = {}

                    def consume(p, P2, psq):
                        for r in (0, 1):
                            kt = 2 * p + r
                            c0 = max(0, 128 * (kt - 4 * j))
                            nc.tensor.matmul(
                                acc_y[:, c0:512],
                                V_all[:, 128 * kt:128 * kt + 128],
                                P2[:, 512 * r + c0:512 * r + 512],
                                start=(p == 0 and r == 0),
                                stop=(p == npair - 1 and r == 1),
                                skip_group_check=True)
                        # denominator matmul once per 4 k-tiles on the
                        # DVE-summed P (quarters its PE cost); the sum was
                        # emitted a pair ago so PE never waits on it here
                        if psq is not None:
                            t = p // 2
                            nc.tensor.matmul(acc_d[:], onescol_b[:], psq[:],
                                             start=(t == 0), stop=(t == j),
                                             skip_group_check=True)

                    for p in range(npair):
                        ps2 = pw.tile([128, 1024], F32, name="ps2", tag="big",
                                      bufs=2)
                        for r in (0, 1):
                            kt = 2 * p + r
                            nc.tensor.matmul(
                                ps2[:, 512 * r:512 * r + 512],
                                kT[:, 128 * kt:128 * kt + 128],
                                qT[h][:, 512 * j:512 * j + 512],
                                start=True, stop=True, skip_group_check=True)
                        P2 = wk.tile([128, 1024], BF16, name="P2", tag="P",
                                     bufs=4)
                        nc.scalar.activation(P2[:], ps2[:], AF.Exp)
                        if p >= 2 * j:                      # diagonal pair
                            nc.vector.tensor_mul(P2[:], P2[:], M2[p - 2 * j])
                        Ps = wk.tile([128, 512], BF16, name="Ps", tag="Ps",
                                     bufs=4)
                        nc.vector.tensor_add(Ps[:], P2[:, 0:512],
                                             P2[:, 512:1024])
                        psq = None
                        if p % 2 == 1:
                            psq = wk.tile([128, 512], BF16, name="psq",
                                          tag="psq", bufs=2)
                            nc.vector.tensor_add(psq[:], ps_hist.pop(p - 1)[:],
                                                 Ps[:])
                        else:
                            ps_hist[p] = Ps
                        lagged.append((p, P2, psq))
                        if len(lagged) > 1:
                            consume(*lagged.pop(0))
                    while lagged:
                        consume(*lagged.pop(0))
                    rd = wk.tile([1, 512], F32, name="rd", tag="stat", bufs=4)
                    nc.vector.reciprocal_approx_fast(rd[:], acc_d[:])
                    Rd = wk.tile([128, 512], F32, name="Rd", tag="Rb", bufs=2)
                    nc.gpsimd.partition_broadcast(Rd[:], rd[0:1, :])
                    nc.vector.tensor_mul(yT[h][:, 512 * j:512 * j + 512],
                                         acc_y[:], Rd[:])

                # ---- per q-head: q(h+1) projections emitted ahead of
                # attention(h) so PE crosses head boundaries without gaps
                # and ACT's exp stream overlaps projection matmuls.  (All
                # tails must precede the attention blocks: the blocks' PSUM
                # slots are released by tail instructions, and engine FIFOs
                # would deadlock on a later-emitted release.) ----
                pending = run_proj_jobs([("q", 0, j) for j in range(NST)],
                                        pending)
                for h in range(RQ):
                    if h + 1 < RQ:
                        pending = run_proj_jobs(
                            [("q", h + 1, j) for j in range(NST)], pending)
                    while pending:
                        emit_proj_tail(*pending.pop(0))
                    for j in range(NST):
                        attention_block(h, j)

                # ---- output projection (transposed: out^T[D, s], bf16);
                # PSUM->SBUF casts alternate DVE/ACT to split the load ----
                ptags = ["big", "acc", "bcast", "small"]
                pbufs = {"big": 2, "acc": 2, "bcast": 1, "small": 1}
                for dt in range(NDT):
                    osb = wk.tile([128, S], BF16, name="osb", tag="osb")
                    for sjj in range(NST):
                        po = pw.tile([128, 512], F32, name=f"po{sjj}",
                                     tag=ptags[sjj], bufs=pbufs[ptags[sjj]])
                        for h in range(RQ):
                            nc.tensor.matmul(
                                po[:],
                                wpT[:, dt * RQ * 128 + 128 * h:
                                    dt * RQ * 128 + 128 * h + 128],
                                yT[h][:, 512 * sjj:512 * sjj + 512],
                                start=(h == 0), stop=(h == RQ - 1))
                        dst = osb[:, 512 * sjj:512 * sjj + 512]
                        if sjj % 2 == 0:
                            nc.vector.tensor_copy(dst, po[:])
                        else:
                            nc.scalar.copy(dst, po[:])
                    nc.sync.dma_start(outT_d[:, dt * S:(dt + 1) * S], osb[:])

            if n_timing_iters > 1:
                with tc.For_i(0, n_timing_iters, 1):
                    body()
            else:
                body()

    # Steer the ACT-table chooser: the kernel only uses {Copy, Square, Ln,
    # Exp}, all present in natural_log_exp_and_others.  The default greedy
    # pick resolves Ln->natural_log and Exp->exp_and_others, ping-ponging
    # table loads (~2.7us each, stalls PE).  Hiding Ln/Exp from every other
    # set makes the combined set the only candidate, so it loads once.
    _orig_gat = bacc.get_activation_tables
    def _gat_one_set(arch):
        tabs = _orig_gat(arch)
        for name, fns in tabs.items():
            if name != "natural_log_exp_and_others":
                fns.discard(AF.Ln)
                fns.discard(AF.Exp)
        return tabs
    bacc.get_activation_tables = _gat_one_set
    try:
        nc.compile()
    finally:
        bacc.get_activation_tables = _orig_gat
    return nc


def _get_program(n_timing_iters=1):
    key = n_timing_iters
    if key not in _PROG_CACHE:
        _PROG_CACHE[key] = _build_program(n_timing_iters)
    return _PROG_CACHE[key]


def _host_inputs(x, Wq, Wk, Wv, Wproj, q_gain):
    """Build the 8 per-core input maps (host-side layout prep)."""
    half = HD // 2
    inv = 1.0 / (10000.0 ** (np.arange(0, HD, 2, dtype=np.float64) / HD))
    t = np.arange(S, dtype=np.float64)
    fr = np.outer(t, inv).astype(np.float32)          # [S, 64]
    cos = np.cos(fr).astype(np.float32)
    sin = np.sin(fr).astype(np.float32)
    cosF = np.concatenate([cos.T, cos.T], 0).astype(bf)          # [128, S]
    sinF = np.concatenate([sin.T, -sin.T], 0).astype(bf)

    onescol_f = np.ones((128, 1), np.float32)
    onescol_b = np.ones((128, 1), bf)
    ident = np.eye(128, dtype=np.float32)
    triu = (np.arange(128)[None, :] >= np.arange(128)[:, None]).astype(np.float32)
    ones128 = np.ones((128, 128), np.float32)
    zeros128 = np.zeros((128, 128), np.float32)

    def mrel(r):
        return np.concatenate([zeros128] * r + [triu] + [ones128] * (3 - r), 1)

    idm = np.concatenate(
        [ident, mrel(0)[:, 0:512], mrel(1)[:, 0:512],
         mrel(2)[:, 0:512], mrel(3)[:, 0:512]], 1).astype(bf)

    # x^T per batch: [128, (dt, s)] partition-major
    xT = [np.ascontiguousarray(
            x[b].reshape(S, NDT, 128).transpose(2, 1, 0)).reshape(128, NDT * S)
          .astype(bf)
          for b in range(B)]

    in_maps = []
    for c in range(NCORES):
        b, g = c // HKV, c % HKV
        # wq head-major: head h -> [128, (dt, c)] like k/v
        wq_heads = []
        for h in range(RQ):
            whT = Wq[512 * g + 128 * h: 512 * g + 128 * (h + 1)].T  # [D, 128]
            wq_heads.append(
                whT.reshape(NDT, 128, HD).transpose(1, 0, 2).reshape(128, NDT * HD))
        wq = np.ascontiguousarray(np.concatenate(wq_heads, axis=1)).astype(bf)
        wk_ = np.ascontiguousarray(
            Wk[128 * g:128 * (g + 1)].T.reshape(NDT, 128, HD)
            .transpose(1, 0, 2).reshape(128, NDT * HD)).astype(bf)
        wv = np.ascontiguousarray(
            Wv[128 * g:128 * (g + 1)].T.reshape(NDT, 128, HD)
            .transpose(1, 0, 2).reshape(128, NDT * HD)).astype(bf)
        wpT = np.ascontiguousarray(Wproj[:, 512 * g:512 * (g + 1)].T)  # [512, 2048]
        # [c-part 128, (dt, h, m)]: wpT2[c, dt*512+128h+m] = Wp[128dt+m, 512g+128h+c]
        wpT = np.ascontiguousarray(
            wpT.reshape(RQ, 128, NDT, 128).transpose(1, 2, 0, 3).reshape(
                128, NDT * RQ * 128)).astype(bf)
        cfsv = np.zeros((1, 641), np.float32)
        cfsv[0, 0] = EPS
        cfsv[0, 1:129] = 1.0
        gv = (q_gain[RQ * g: RQ * (g + 1)].astype(np.float64)
              / np.sqrt(HD))
        cfsv[0, 129:133] = np.log(np.maximum(gv, 1e-30)).astype(np.float32)
        in_maps.append({
            "xT": xT[b],
            "wqT": wq,
            "wkT": wk_,
            "wvT": wv,
            "wpT": wpT,
            "cosF": cosF, "sinF": sinF, "cfs": cfsv,
            "onescol_f": onescol_f, "onescol_b": onescol_b, "idm": idm,
        })
    return in_maps


def kernel(x, Wq, Wk, Wv, Wproj, q_gain, _n_timing_iters=1, _return_raw=False):
    x = np.asarray(x, np.float32)
    in_maps = _host_inputs(np.asarray(x, np.float32),
                           np.asarray(Wq, np.float32),
                           np.asarray(Wk, np.float32),
                           np.asarray(Wv, np.float32),
                           np.asarray(Wproj, np.float32),
                           np.asarray(q_gain, np.float32))
    nc = _get_program(_n_timing_iters)
    res = run_bass_kernel_spmd(nc, in_maps, core_ids=list(range(NCORES)),
                               trace=False)
    if _return_raw:
        return res
    out = np.zeros((B, S, D), np.float32)
    for c in range(NCORES):
        b = c // HKV
        outT = res.results[c]["outT"].astype(np.float32).reshape(
            128, NDT, S).transpose(1, 0, 2).reshape(D, S)
        out[b] += outT.T
    return out


if __name__ == "__main__":
    rng = np.random.default_rng(0)
    x = rng.standard_normal((B, S, D)).astype(np.float32)
    Wq = (rng.standard_normal((D, D)) * 0.02).astype(np.float32)
    Wk = (rng.standard_normal((512, D)) * 0.02).astype(np.float32)
    Wv = (rng.standard_normal((512, D)) * 0.02).astype(np.float32)
    Wp = (rng.standard_normal((D, D)) * 0.02).astype(np.float32)
    g = np.ones(H, np.float32)
    out = kernel(x, Wq, Wk, Wv, Wp, g)
    print("out", out.shape, out.dtype, float(np.abs(out).max()))
